# revision 46
# baseline (speedup 1.0000x reference)
"""Trainium2 Bass kernel for nn_Attention_18949395710608 (v15).

Multi-head causal self-attention, B=4, S=2048, D=1024, H=16, dk=dv=64.

Sharding: 8 cores = 4 batches x 2 head-groups (8 heads each).
Each core computes a partial output projection over its 8 heads for its
batch; the host sums the two partials per batch (the "all-reduce").

Trace-driven optimization history: 299.7us (v2 baseline) -> ~264.5us.
Key mechanisms (all verified against perfetto traces of real HW runs):
  - Host-preformatted inputs: every DMA moves [128 x 2-8KB-contiguous]
    rows (fat descriptors), split across the three DMA queues
    (sync/scalar HWDGE + gpsimd SWDGE) in consumption order. Cold loads
    (wqk2/3, wo) are emitted mid-schedule so the startup crunch only
    moves the ~5.6MB needed before attention starts.
  - 26 dataless warmup matmuls ramp the PE clock (0.65 -> 2.4GHz takes
    ~3us of continuous work) while the first xt chunk is still in
    flight; the pair0 Q/K prologue runs kt-major with 8 open PSUM
    accumulators so the PE rides the DMA arrival front.
  - Score matmuls emitted in bursts of 2 tiles (pairs that follow a
    score pair stall ~110ns; bursting pays the tax once per group).
  - AV trails SC by ~6 tiles so probs are always ready; V-projection
    fins tracked per-seq-tile (vdone) to release the AV backlog early.
  - exp split ~50:50 between ScalarE (table exp) and DVE (Schraudolph
    int16 bit-trick, ~1.8% rms on those tiles), 2:1 toward ScalarE in
    pair3's copy-free stretches; diagonal tiles on ScalarE with the
    triangular-mask multiply on DVE (short latency, gates diag AV).
  - Softmax denominators ride the 65th V row; reciprocals broadcast
    across partitions via a DRAM bounce on the sync queue, which
    carries nothing else mid-kernel (output writes ride gpsimd;
    final four chains use sync to shorten the drain); normalize muls
    are deferred and run on GpSimd.
  - pair3's own Q/K chains are created in the order its j-descending
    attention consumes them and woven into its otherwise fillerless
    chunks (pair2 reserves 40 filler ops for this); output-projection
    chains are staged as their OT dependencies complete, with
    flush_prev_runs() guaranteeing the normalize muls are emitted
    before any chain that reads them.
  - Output is written bf16 (the host partial-sum upcasts to f32);
    rel err ~7.2e-3 vs the 2e-2 gate.
"""

import math
from collections import deque

import numpy as np
import ml_dtypes

B, S, D, H, DK = 4, 2048, 1024, 16, 64
HL = H // 2          # heads per core
HDL = HL * DK        # 512 local head dims
P = 128
NKT = D // P         # 8 k-tiles over d_in
NPT = HDL // P       # 4 partition tiles over local head dims (head pairs)
NST = S // P         # 16 seq tiles
QC = 512             # query chunk
NQC = S // QC        # 4 query chunks
SCALE = 1.0 / math.sqrt(DK)

AV_LAG = 6           # target AV-behind-SC distance (tiles)
ALPHA = 128.0 / math.log(2.0)
BETA = 128.0 * (127.0 - 0.0579)

BF16 = ml_dtypes.bfloat16

_CACHED = {}


def _build_nc():
    import concourse.bass as bass
    import concourse.bacc as bacc
    import concourse.tile as tile
    from concourse import mybir

    bf = mybir.dt.bfloat16
    f32 = mybir.dt.float32
    i16 = mybir.dt.int16

    nc = bacc.Bacc(None, target_bir_lowering=False)

    # host-preformatted, per-partition-contiguous layouts
    xT_d = nc.dram_tensor("xT", [P, NKT, S], bf, kind="ExternalInput")
    wqk_d = nc.dram_tensor("wqk", [P, NPT, 2, NKT, P], bf, kind="ExternalInput")
    wv_d = nc.dram_tensor("wv", [P, NKT, HDL], bf, kind="ExternalInput")
    wo_d = nc.dram_tensor("wo", [P, NPT, D], bf, kind="ExternalInput")
    mask_d = nc.dram_tensor("mask", [P, 2 * P], bf, kind="ExternalInput")
    out_d = nc.dram_tensor("out", [S, D], bf, kind="ExternalOutput")

    out_v = out_d[:, :].rearrange("(t p) n -> p t n", p=P)

    with tile.TileContext(nc) as tc:
        with (
            tc.tile_pool(name="consts", bufs=1) as consts,
            tc.tile_pool(name="probs", bufs=18) as ppool,
            tc.tile_pool(name="small", bufs=2) as spool,
            tc.tile_pool(name="osb", bufs=7) as opool,
            tc.tile_pool(name="avst", bufs=3) as apool,
            tc.tile_pool(name="dramp", bufs=4, space="DRAM") as dramp,
        ):
            # ---- persistent tiles ----
            xt_all = consts.tile([P, NKT, S], bf, name="xt_all")
            wqk_all = consts.tile([P, NPT, 2, NKT, P], bf, name="wqk_all")
            wv_all = consts.tile([P, NKT, HDL], bf, name="wv_all")
            wo_sb = consts.tile([P, NPT, D], bf)
            mask_sb = consts.tile([P, 2, P], bf)
            QT_sb = consts.tile([P, NPT, S], bf)
            KT_sb = consts.tile([P, NPT, S], bf)
            V_sb = consts.tile([P, NST, HL, 66], bf)
            OT_t = [
                [consts.tile([P, QC], bf, name=f"ot{p}_{j}") for j in range(NQC)]
                for p in range(NPT)
            ]

            # ---- input DMAs: 3 queues, consumption-ordered kt chunks.
            # prologue consumes kt chunks at ~1.7us apiece from ~13us.
            # ~4-deep queue limit: late calls stall the issuing sequencer,
            # so urgent loads go first and cold ones (wqk2/3, wo) last.
            # scalar queue (HWDGE; emitted before the exp-table warm):
            nc.scalar.dma_start(out=xt_all[:, 1, :], in_=xT_d[:, 1, :])
            nc.scalar.dma_start(out=xt_all[:, 3, :], in_=xT_d[:, 3, :])
            nc.scalar.dma_start(out=xt_all[:, 6, :], in_=xT_d[:, 6, :])
            nc.scalar.dma_start(out=wv_all[:, 0:4, :], in_=wv_d[:, 0:4, :])
            # sync queue (HWDGE): xt0 first so the first boot matmul fires
            # as early as possible (wq0/wk0 ride the gpsimd queue, which
            # starts later but only carries small tiles first)
            nc.sync.dma_start(out=xt_all[:, 0, :], in_=xT_d[:, 0, :])
            nc.sync.dma_start(out=xt_all[:, 2, :], in_=xT_d[:, 2, :])
            nc.sync.dma_start(out=xt_all[:, 5, :], in_=xT_d[:, 5, :])
            nc.sync.dma_start(out=xt_all[:, 7, :], in_=xT_d[:, 7, :])
            # gpsimd queue (SWDGE). Cold loads (wqk2/3, wo) are deferred
            # into the schedule so the startup crunch only moves what is
            # urgently needed.
            nc.gpsimd.dma_start(
                out=mask_sb[:, :, :],
                in_=mask_d[:, :].rearrange("p (a c) -> p a c", a=2),
            )
            nc.gpsimd.dma_start(
                out=wqk_all[:, 0, 0, :, :], in_=wqk_d[:, 0, 0, :, :]
            )  # wq pair0
            nc.gpsimd.dma_start(
                out=wqk_all[:, 0, 1, :, :], in_=wqk_d[:, 0, 1, :, :]
            )  # wk pair0
            nc.gpsimd.dma_start(out=xt_all[:, 4, :], in_=xT_d[:, 4, :])
            nc.gpsimd.dma_start(
                out=wqk_all[:, 1, :, :, :], in_=wqk_d[:, 1, :, :, :]
            )
            nc.gpsimd.dma_start(out=wv_all[:, 4:8, :], in_=wv_d[:, 4:8, :])

            # warm the exp activation table (after the scalar-queue DMAs)
            scr = consts.tile([1, 8], f32)
            nc.vector.memset(scr[:, :], 0.0)
            scr2 = consts.tile([1, 8], f32)
            nc.scalar.activation(
                out=scr2[:, :], in_=scr[:, :],
                func=mybir.ActivationFunctionType.Exp, scale=1.0,
            )

            nc.vector.memset(V_sb[:, :, :, 64:65], 1.0)

            # persistent epilogue scratch (base-0 partitions for broadcast)
            recin_t = [consts.tile([33, QC], f32, name=f"rcin{h}") for h in range(2)]
            recful_t = [consts.tile([33, QC], f32, name=f"rcfl{h}") for h in range(2)]
            for h in range(2):
                nc.vector.memset(recin_t[h][:, :], 1.0)

            copy_alt = [0]

            def psum_copy(dst_ap, src_ap):
                # alternate psum->sbuf copies between ScalarE and DVE
                copy_alt[0] ^= 1
                if copy_alt[0]:
                    nc.scalar.copy(dst_ap, src_ap)
                else:
                    nc.vector.tensor_copy(dst_ap, src_ap)

            # warmup operands for the PE p-state ramp (contents irrelevant)
            warm_w = consts.tile([P, P], bf, name="warm_w")
            warm_x = consts.tile([P, QC], bf, name="warm_x")
            nc.vector.memset(warm_w[:, :], 0.0)
            nc.vector.memset(warm_x[:, :], 0.0)

            # ---- prologue: pair0 Q/K projections, kt-major so the PE is
            # paced by the xt kt-chunk DMA arrivals (8 open accumulators).
            with tc.tile_pool(name="ps_boot", bufs=1, space="PSUM") as ps_boot:
                bootq = [
                    ps_boot.tile([P, QC], f32, name=f"bq{sc}") for sc in range(NQC)
                ]
                bootk = [
                    ps_boot.tile([P, QC], f32, name=f"bk{sc}") for sc in range(NQC)
                ]
                # dataless warmup matmuls: keep the PE busy from ~6.5us so
                # the clock is at 2.4GHz when the first xt chunk lands
                for _ in range(26):
                    nc.tensor.matmul(
                        bootq[0][:, :], lhsT=warm_w[:, :], rhs=warm_x[:, :],
                        start=True, stop=True,
                    )
                for kt in range(NKT):
                    last = kt == NKT - 1
                    for sc in range(NQC):
                        nc.tensor.matmul(
                            bootq[sc][:, :],
                            lhsT=wqk_all[:, 0, 0, kt, :],
                            rhs=xt_all[:, kt, sc * QC : (sc + 1) * QC],
                            start=(kt == 0),
                            stop=last,
                        )
                        if last:  # fin as soon as this sc's accum closes
                            psum_copy(
                                QT_sb[:, 0, sc * QC : (sc + 1) * QC],
                                bootq[sc][:, :],
                            )
                    for sc in range(NQC):
                        nc.tensor.matmul(
                            bootk[sc][:, :],
                            lhsT=wqk_all[:, 0, 1, kt, :],
                            rhs=xt_all[:, kt, sc * QC : (sc + 1) * QC],
                            start=(kt == 0),
                            stop=last,
                        )
                        if last:
                            psum_copy(
                                KT_sb[:, 0, sc * QC : (sc + 1) * QC],
                                bootk[sc][:, :],
                            )

            with (
                tc.tile_pool(name="ps_sc", bufs=2, space="PSUM") as ps_sc,
                tc.tile_pool(name="ps_av", bufs=1, space="PSUM") as ps_av,
                tc.tile_pool(name="ps_pj", bufs=2, space="PSUM") as ps_pj,
            ):
                # ---- filler machinery: single-MM granularity proj work ----
                filler = deque()  # items: (tag, closure)
                remaining = {}    # tag -> ops left in queue
                vdone = set()     # st indices whose V fin has been emitted

                def _push(tag, fn):
                    filler.append((tag, fn))
                    remaining[tag] = remaining.get(tag, 0) + 1

                def add_qk_chain(qk, dst, pair, sc, tag):
                    st8 = {}

                    def mk(kt):
                        def f():
                            if kt == 0:
                                st8["ps"] = ps_pj.tile(
                                    [P, QC], f32, tag="pj", name="pj"
                                )
                            nc.tensor.matmul(
                                st8["ps"][:, :],
                                lhsT=wqk_all[:, pair, qk, kt, :],
                                rhs=xt_all[:, kt, sc * QC : (sc + 1) * QC],
                                start=(kt == 0),
                                stop=(kt == NKT - 1),
                            )
                        return f

                    def fin():
                        psum_copy(
                            dst[:, pair, sc * QC : (sc + 1) * QC], st8["ps"][:, :]
                        )

                    for kt in range(NKT):
                        _push(tag, mk(kt))
                    _push(tag, fin)

                def add_v_chain(st):
                    st8 = {}

                    def mk(kt):
                        def f():
                            if kt == 0:
                                st8["ps"] = ps_pj.tile(
                                    [P, QC], f32, tag="pj", name="pj"
                                )
                            nc.tensor.matmul(
                                st8["ps"][:, :],
                                lhsT=xt_all[:, kt, st * P : (st + 1) * P],
                                rhs=wv_all[:, kt, :],
                                start=(kt == 0),
                                stop=(kt == NKT - 1),
                            )
                        return f

                    def fin():
                        nc.vector.tensor_copy(
                            V_sb[:, st, :, 0:64],
                            st8["ps"][:, :].rearrange("p (h d) -> p h d", h=HL),
                        )
                        vdone.add(st)

                    for kt in range(NKT):
                        _push("v", mk(kt))
                    _push("v", fin)

                def add_o_chain(st, nch, qsync=False):
                    st8 = {}

                    def mk(p):
                        def f():
                            if p == 0:
                                st8["ps"] = ps_pj.tile(
                                    [P, QC], f32, tag="pj", name="pj"
                                )
                            nc.tensor.matmul(
                                st8["ps"][:, :],
                                lhsT=OT_t[p][st // 4][
                                    :, (st % 4) * P : (st % 4 + 1) * P
                                ],
                                rhs=wo_sb[:, p, nch * QC : (nch + 1) * QC],
                                start=(p == 0),
                                stop=(p == NPT - 1),
                            )
                        return f

                    def fin():
                        osb = opool.tile([P, QC], bf, tag="osb", name="osb")
                        psum_copy(osb[:, :], st8["ps"][:, :])
                        eng = nc.sync if qsync else nc.gpsimd
                        eng.dma_start(
                            out=out_v[:, st, nch * QC : (nch + 1) * QC],
                            in_=osb[:, :],
                        )

                    for p in range(NPT):
                        _push("o", mk(p))
                    _push("o", fin)

                def _pop_one():
                    tag, fn = filler.popleft()
                    remaining[tag] -= 1
                    fn()

                def emit_filler(n, reserve=0):
                    k = 0
                    while filler and k < n and len(filler) > reserve:
                        _pop_one()
                        k += 1

                def drain_filler():
                    while filler:
                        _pop_one()

                def drain_tag(tag):
                    while remaining.get(tag, 0) > 0:
                        _pop_one()

                # ---- attention emission ----
                av_fifo = deque()
                av_tiles = {}
                deferred = deque()
                tile_ctr = [0]
                run_id = [0]
                gp_muls = [False]  # route normalize muls to GpSimd at tail
                scalar_heavy = [False]

                def emit_sc(pair, j, kt, nkt):
                    a = kt - 4 * j
                    off = P * a if a >= 0 else 0
                    scp = ps_sc.tile([P, 2 * QC], f32, tag="scp", name="scp")
                    for h01 in range(2):
                        base = 64 * h01
                        nc.tensor.matmul(
                            scp[:, h01 * QC + off : (h01 + 1) * QC],
                            lhsT=KT_sb[base : base + 64, pair, kt * P : (kt + 1) * P],
                            rhs=QT_sb[
                                base : base + 64, pair, j * QC + off : (j + 1) * QC
                            ],
                            start=True,
                            stop=True,
                        )
                    pb = ppool.tile([P, 2, QC], bf, tag="pb", name="pb")
                    tile_ctr[0] += 1
                    # split exp between ScalarE (table exp) and DVE
                    # (Schraudolph bit-trick); diagonal tiles -> ScalarE.
                    # 50:50 normally; 2:1 toward ScalarE when it carries no
                    # psum copies (pair3's fillerless first chunks).
                    if scalar_heavy[0]:
                        use_dve = (a < 0) and (tile_ctr[0] % 3 == 2)
                    else:
                        use_dve = (a < 0) and (tile_ctr[0] % 2 == 0)
                    if off:
                        pbv = pb[:, :, off:QC]
                        scv = scp[:, :].rearrange("p (h q) -> p h q", h=2)[
                            :, :, off:QC
                        ]
                    else:
                        pbv = pb[:, :, :].rearrange("p h q -> p (h q)")
                        scv = scp[:, :]
                    if use_dve:
                        nc.vector.tensor_scalar(
                            out=pbv.bitcast(i16),
                            in0=scv,
                            scalar1=SCALE * ALPHA,
                            scalar2=BETA,
                            op0=mybir.AluOpType.mult,
                            op1=mybir.AluOpType.add,
                        )
                    else:
                        nc.scalar.activation(
                            out=pbv, in_=scv,
                            func=mybir.ActivationFunctionType.Exp, scale=SCALE,
                        )
                    if a >= 0:
                        # DVE: short latency matters, the diagonal AV waits
                        nc.vector.tensor_mul(
                            pb[:, :, off : off + P],
                            pb[:, :, off : off + P],
                            mask_sb[:, :, :],
                        )
                    av_fifo.append((pair, j, kt, nkt, off, pb, run_id[0]))

                def emit_av(unit):
                    pair, j, kt, nkt, off, pb, _rid = unit
                    if kt == 0:
                        av_tiles[0] = ps_av.tile([65, QC], f32, name="av0")
                        av_tiles[1] = ps_av.tile([65, QC], f32, name="av1")
                    for h01 in range(2):
                        nc.tensor.matmul(
                            av_tiles[h01][0:65, off:QC],
                            lhsT=V_sb[:, kt, 2 * pair + h01, 0:65],
                            rhs=pb[:, h01, off:QC],
                            start=(kt == 0),
                            stop=(kt == nkt - 1),
                        )
                    if kt == nkt - 1:
                        epilogue(pair, j)

                def epilogue(pair, j):
                    # Stage av out of PSUM immediately; reciprocals bounce
                    # through DRAM (sync queue; output writes are on gpsimd
                    # so the bounce never queues behind them); the final
                    # normalize muls are deferred.
                    av0, av1 = av_tiles[0], av_tiles[1]
                    avs = apool.tile([P, QC], f32, tag="avs", name="avs")
                    nc.scalar.copy(avs[0:64, :], av0[0:64, :])
                    nc.vector.tensor_copy(avs[64:128, :], av1[0:64, :])
                    nc.vector.tensor_copy(recin_t[0][0:1, :], av0[64:65, :])
                    nc.vector.tensor_copy(recin_t[1][0:1, :], av1[64:65, :])
                    rd = dramp.tile([2, QC], f32, tag="rd", name="rd")
                    for h01 in range(2):
                        nc.vector.reciprocal_approx_fast(
                            out=recful_t[h01][0:33, :], in_=recin_t[h01][0:33, :]
                        )
                        nc.sync.dma_start(
                            out=rd[h01 : h01 + 1, :], in_=recful_t[h01][0:1, :]
                        )
                    bcs = spool.tile([P, QC], f32, tag="bcs", name="bcs")
                    for h01 in range(2):
                        bsrc = bass.AP(
                            tensor=rd.tensor,
                            offset=rd[h01 : h01 + 1, :].offset,
                            ap=[[0, 64], [1, QC]],
                        )
                        nc.sync.dma_start(
                            out=bcs[64 * h01 : 64 * h01 + 64, :], in_=bsrc
                        )
                    def muls():
                        # SBUF-only op: GpSimd (ScalarE/DVE are the scarce
                        # engines; GpSimd only issues out-write DMAs)
                        nc.gpsimd.tensor_mul(
                            OT_t[pair][j][0:64, :], avs[0:64, :], bcs[0:64, :]
                        )
                        nc.gpsimd.tensor_mul(
                            OT_t[pair][j][64:128, :], avs[64:128, :], bcs[64:128, :]
                        )

                    deferred.append(muls)

                def pop_avs():
                    # keep AV roughly AV_LAG tiles behind SC; only consume
                    # V_sb tiles whose V-projection fin has been emitted
                    def ready():
                        return av_fifo and av_fifo[0][2] in vdone

                    # hard-drain anything older than the previous SC run
                    while ready() and av_fifo[0][6] <= run_id[0] - 2:
                        emit_av(av_fifo.popleft())
                    pops = 0
                    if len(av_fifo) > 6 or (
                        av_fifo and av_fifo[0][6] < run_id[0]
                    ):
                        pops = 4
                    elif len(av_fifo) > AV_LAG:
                        pops = 2
                    for _ in range(pops):
                        if not ready():
                            break
                        emit_av(av_fifo.popleft())

                def flush_prev_runs():
                    # emit every AV of completed runs plus all pending
                    # normalize muls, so chains reading OT can be emitted
                    while av_fifo and av_fifo[0][6] <= run_id[0] - 1:
                        emit_av(av_fifo.popleft())
                    while deferred:
                        deferred.popleft()()

                # ---- schedule ----
                # V chains + QK(pair1) become filler woven into attention
                for st in range(NST):
                    add_v_chain(st)
                for sc in range(NQC):
                    add_qk_chain(0, QT_sb, 1, sc, "qk1")
                    add_qk_chain(1, KT_sb, 1, sc, "qk1")

                j_orders = {0: [0, 1, 2, 3], 1: [0, 1, 2, 3],
                            2: [0, 1, 2, 3], 3: [3, 2, 1, 0]}
                for pair in range(NPT):
                    if pair == 1:
                        for sc in range(NQC):
                            add_qk_chain(0, QT_sb, 2, sc, "qk2")
                            add_qk_chain(1, KT_sb, 2, sc, "qk2")
                    elif pair == 2:
                        # pair3's chains, created in the order pair3's
                        # j-descending attention consumes them so they can
                        # weave into its otherwise fillerless chunks
                        add_qk_chain(0, QT_sb, 3, 3, "p3q3")
                        for b in range(NQC):
                            add_qk_chain(1, KT_sb, 3, b, f"p3k{b}")
                        for scq in (2, 1, 0):
                            add_qk_chain(0, QT_sb, 3, scq, f"p3q{scq}")
                    # this pair's QT/KT chains must be fully emitted before
                    # its score matmuls read them
                    if pair in (1, 2):
                        drain_tag(f"qk{pair}")
                    elif pair == 3:
                        drain_tag("p3q3")
                        drain_tag("p3k0")
                    for jj, j in enumerate(j_orders[pair]):
                        nkt = 4 * j + 4
                        # deferred cold loads: emit each ~50us before its
                        # first consumer so it misses the startup crunch
                        if j == 2 and pair == 0:
                            nc.gpsimd.dma_start(
                                out=wqk_all[:, 2, :, :, :], in_=wqk_d[:, 2, :, :, :]
                            )
                        elif j == 2 and pair == 1:
                            nc.gpsimd.dma_start(
                                out=wqk_all[:, 3, :, :, :], in_=wqk_d[:, 3, :, :, :]
                            )
                        elif j == 2 and pair == 2:
                            nc.gpsimd.dma_start(out=wo_sb[:, :, :], in_=wo_d[:, :, :])
                        if pair == 3:
                            scalar_heavy[0] = jj < 2
                            if jj >= 2:
                                gp_muls[0] = True
                            # stage pair3's remaining Q/K fins and the
                            # output-projection groups as their OT deps
                            # complete (j descending)
                            if jj == 1:
                                drain_tag("p3q2")
                            elif jj == 2:
                                drain_tag("p3q1")
                                flush_prev_runs()
                                for st in range(12, 16):
                                    add_o_chain(st, 1)
                                for st in range(8, 12):
                                    add_o_chain(st, 0)
                            elif jj == 3:
                                drain_tag("p3q0")
                                flush_prev_runs()
                                for st in range(8, 12):
                                    add_o_chain(st, 1)
                        # SC tiles in bursts of 2 (consecutive score pairs
                        # pipeline at full rate; isolated ones stall ~110ns)
                        for kt2 in range(0, nkt, 2):
                            if pair == 3 and jj == 0:
                                if kt2 == 4:
                                    drain_tag("p3k1")
                                elif kt2 == 8:
                                    drain_tag("p3k2")
                                elif kt2 == 12:
                                    drain_tag("p3k3")
                            elif pair == 3 and jj == 1 and kt2 == 4:
                                # emit j3's muls now; the o-chains that
                                # read them are added two groups later so
                                # the reciprocal bounce latency is hidden
                                flush_prev_runs()
                            elif pair == 3 and jj == 1 and kt2 == 8:
                                for st in range(12, 16):
                                    add_o_chain(st, 0)
                            pop_avs()
                            if pair == 0 and j < 2:
                                emit_filler(24)
                            elif pair == 2:
                                emit_filler(4, reserve=40)
                            else:
                                emit_filler(4)
                            emit_sc(pair, j, kt2, nkt)
                            emit_sc(pair, j, kt2 + 1, nkt)
                            if deferred:
                                deferred.popleft()()
                        run_id[0] += 1
                # drain remaining AV work, then final output projections
                while av_fifo:
                    emit_av(av_fifo.popleft())
                while deferred:
                    deferred.popleft()()
                drain_filler()
                for st in range(4, 8):
                    add_o_chain(st, 0)
                    add_o_chain(st, 1)
                for st in range(0, 4):
                    add_o_chain(st, 0, qsync=(st >= 2))
                    add_o_chain(st, 1, qsync=(st >= 2))
                drain_filler()

    nc.compile()
    return nc


def get_nc(debug=False):
    key = ("nc", debug)
    if key not in _CACHED:
        _CACHED[key] = _build_nc()
    return _CACHED[key]


def make_core_inputs(x, W_q, W_k, W_v, W_o):
    """Per-core input dicts (numpy, bf16, per-partition-contiguous)."""
    tri = np.triu(np.ones((P, P), np.float32))  # c>=r -> 1
    mask_np = np.concatenate([tri, tri], axis=1).astype(BF16)  # (P, 2P)

    def fmt_pkt(W):  # [D, HDL] -> [P, NPT, NKT, P]
        return W.reshape(NKT, P, NPT, P).transpose(1, 2, 0, 3)

    in_maps = []
    for c in range(8):
        b, g = c // 2, c % 2
        hs = slice(g * HL, (g + 1) * HL)
        Wq_l = W_q[hs].transpose(1, 0, 2).reshape(D, HDL)
        Wk_l = W_k[hs].transpose(1, 0, 2).reshape(D, HDL)
        Wv_l = W_v[hs].transpose(1, 0, 2).reshape(D, HDL)
        Wo_l = W_o[hs].reshape(HDL, D)
        wqk = np.stack([fmt_pkt(Wq_l), fmt_pkt(Wk_l)], axis=2)  # [P,NPT,2,NKT,P]
        in_maps.append(
            {
                # x[b] [S, D] -> [P, NKT, S]
                "xT": np.ascontiguousarray(
                    np.asarray(x[b]).reshape(S, NKT, P).transpose(2, 1, 0)
                ).astype(BF16),
                "wqk": np.ascontiguousarray(wqk).astype(BF16),
                # [D, HDL] -> [P, NKT, HDL]
                "wv": np.ascontiguousarray(
                    Wv_l.reshape(NKT, P, HDL).transpose(1, 0, 2)
                ).astype(BF16),
                # [HDL, D] -> [P, NPT, D]
                "wo": np.ascontiguousarray(
                    Wo_l.reshape(NPT, P, D).transpose(1, 0, 2)
                ).astype(BF16),
                "mask": mask_np,
            }
        )
    return in_maps


def kernel(x, mask, W_q, W_k, W_v, W_o):
    from concourse.bass_utils import run_bass_kernel_spmd

    x = np.asarray(x, np.float32)
    nc = get_nc()
    in_maps = make_core_inputs(
        x, np.asarray(W_q), np.asarray(W_k), np.asarray(W_v), np.asarray(W_o)
    )
    res = run_bass_kernel_spmd(nc, in_maps, core_ids=list(range(8)))
    out = np.zeros((B, S, D), np.float32)
    for c in range(8):
        out[c // 2] += np.asarray(res.results[c]["out"], np.float32)
    return out
